# revision 27
# baseline (speedup 1.0000x reference)
"""Trainium2 Bass kernel for BinarizedLinear: y = x @ sign(W)^T.

Full-input contract: kernel(x, W) takes the unsharded inputs
(x: [8192, 4096] f32, W: [4096, 4096] f32) and returns y: [8192, 4096] f32.

Distribution: data-parallel over tokens. Each of the 8 NeuronCores gets a
[1024, 4096] token shard of x plus a full replica of W, computes
y_shard = x_shard @ sign(W)^T, and the shards are concatenated on the host.

Device kernel (per core):
  - sign(W) is computed on the Scalar (ACT) engine; the {-1, 0, +1} values
    are exact in fp16, so the matmul runs at the 16-bit TensorE rate
    (4x the fp32 rate). The only lossy step is x's f32->f16 rounding
    (~2^-11 relative per element), done while marshaling the shard.
  - Matmuls contract over in_features (on SBUF partitions), accumulating
    32 k-tiles into PSUM in fp32. The first out-feature block uses all 8
    PSUM banks; later blocks use 4+4 so one group's accumulation overlaps
    the other's drain. Junk matmuls during the data-less startup window
    warm the PE's HAM clock gate to 2.4GHz before real work arrives.
  - Host supplies transposed layouts (x^T per shard in fp16, W^T in
    o-block-major [OB, I, 512] bf16) so every DMA is a single linear
    transfer and the contraction dim lands on SBUF partitions with no
    on-device transposes. DMA engine choice (sync vs scalar HWDGE queues)
    plus pool-gated prefetch depth order HBM traffic by need.

Measured: 462.9us on hardware vs the 437.4us fp16 TensorE roofline for
this shape (94.5%); steady-state matmul cadence is the theoretical 216ns.
"""

import numpy as np

TOKENS, IN_F, OUT_F = 8192, 4096, 4096
N_CORES = 8

LAST_RESULTS = None  # BassKernelResults of the most recent run (for profiling)
_NC_CACHE = {}


def _build_nc(T=TOKENS // N_CORES, I=IN_F, O=OUT_F, o_block=512, t_sub=4):
    """Build + compile the per-core Bass module.

    DRAM tensors (per core):
      xt:  [I, T] f16            -- x_shard^T (compute precision)
      wtb: [OB, I, o_block] bf16 -- W^T, o-block-major (sign-exact wire)
      y:   [T, O] f32
    """
    import concourse.mybir as mybir
    import concourse.tile as tile
    from concourse import bacc

    f32, f16 = mybir.dt.float32, mybir.dt.float16
    bf16 = mybir.dt.bfloat16

    P = 128
    KT = I // P          # k-tiles (contraction)
    OB = O // o_block    # output-feature blocks
    TT = T // P          # token tiles
    assert I % P == 0 and O % o_block == 0 and T % P == 0 and TT % t_sub == 0

    nc = bacc.Bacc(
        "TRN2", target_bir_lowering=False, debug=False, enable_asserts=False
    )
    # x^T travels in fp16 — the kernel's compute precision. The host-side
    # f32->f16 rounding is identical to the f32->f16 cast the device would
    # otherwise perform on arrival (same RNE), so device math and output
    # are bit-identical; shipping the compute format halves x DMA.
    xt = nc.dram_tensor("xt", [I, T], f16, kind="ExternalInput")
    # W^T travels as bf16: bf16 keeps f32's exponent range, so the cast
    # preserves sign exactly (no nonzero value rounds to zero); only sign(W)
    # is consumed, so this is a lossless encoding of the used information.
    wtb = nc.dram_tensor("wtb", [OB, I, o_block], bf16, kind="ExternalInput")
    y = nc.dram_tensor("y", [T, O], f32, kind="ExternalOutput")

    xt3 = xt.ap().rearrange("(k p) t -> k p t", p=P)       # [KT, 128, T]
    wt4 = wtb.ap().rearrange("b (k p) o -> b k p o", p=P)  # [OB, KT, 128, o_block]
    y3 = y.ap().rearrange("(t p) o -> t p o", p=P)         # [TT, 128, O]

    with tile.TileContext(nc) as tc:
        with (
            tc.tile_pool(name="xres", bufs=KT) as xres_pool,
            tc.tile_pool(name="wstage", bufs=6) as wstage_pool,
            tc.tile_pool(name="wb", bufs=KT + 16) as wb_pool,
            tc.tile_pool(name="ystage", bufs=6) as ystage_pool,
            tc.tile_pool(name="psum", bufs=8, space="PSUM") as psum_pool,
        ):
            xf = [None] * KT
            wb = [None] * KT

            def load_x(k):
                xx = xres_pool.tile([P, T], f16, tag="xres", name=f"xf_{k}")
                nc.sync.dma_start(xx[:], xt3[k])
                xf[k] = xx

            def load_w(ob, k):
                st = wstage_pool.tile([P, o_block], bf16, tag="wstage",
                                      name=f"ws_{ob}_{k}")
                # DMA engine choice orders HBM traffic by need: blocks 0/1
                # ride sync (block 0 interleaved with x, block 1 queued
                # behind x — deprioritized for exactly block 0's
                # bandwidth-critical window); blocks 2+ ride the Activation
                # engine's independent HWDGE queue set, with their prefetch
                # depth gated by the wb pool (KT+16 slots) so they cannot
                # creep into block 0's window either.
                dma_eng = nc.sync if ob <= 1 else nc.scalar
                dma_eng.dma_start(st[:], wt4[ob, k])
                wbk = wb_pool.tile([P, o_block], f16, tag="wb", name=f"wb_{ob}_{k}")
                nc.scalar.sign(wbk[:], st[:])
                wb[k] = wbk

            def mm_group(ob, t0, nt, first_ps=None):
                """Accumulate + drain output tiles for t-tiles t0..t0+nt-1."""
                osl = slice(ob * o_block, (ob + 1) * o_block)
                psums = [
                    first_ps if (t == 0 and first_ps is not None) else
                    psum_pool.tile([P, o_block], f32, tag="ps",
                                   name=f"ps_{ob}_{t0 + t}")
                    for t in range(nt)
                ]
                for k in range(KT):
                    for t in range(nt):
                        ti = t0 + t
                        nc.tensor.matmul(
                            psums[t][:],
                            xf[k][:, ti * P:(ti + 1) * P],  # lhsT [K, M]
                            wb[k][:],                        # rhs  [K, N]
                            start=(k == 0),
                            stop=(k == KT - 1),
                        )
                last = (ob == OB - 1) and (t0 + nt == TT)
                for t in range(nt):
                    ti = t0 + t
                    yt = ystage_pool.tile([P, o_block], f32, tag="ystage",
                                          name=f"yt_{ob}_{ti}")
                    # Final group: split drains across DVE and ACT so the
                    # kernel tail isn't serialized on one engine.
                    if last and t % 2 == 1:
                        nc.scalar.copy(yt[:], psums[t][:])
                    else:
                        nc.vector.tensor_copy(yt[:], psums[t][:])
                    nc.sync.dma_start(y3[ti][:, osl], yt[:])

            # With fp16 x on the wire, block 0 is PE-bound, so matmuls that
            # run at the cold 1.2GHz HAM clock cost end-to-end time. Warm
            # the clock gate during the data-less startup window (~4-9us)
            # with junk matmuls on a zeroed tile; they land in the first
            # group's first PSUM bank, which the real k=0 matmul's
            # start=True resets.
            warm_in = wb_pool.tile([P, P], f16, tag="warm", bufs=1,
                                   name="warm_in")
            nc.gpsimd.memset(warm_in[:], 0.0)
            # 30 junk matmuls x ~107ns cold = ~3.2us of PE activity ending
            # right as the first real tiles land (~10.5us): enough to flip
            # the HAM gate warm without the junk queue delaying real work
            # (64 was measured to overshoot data-ready by ~3.3us).
            warm_ps = psum_pool.tile([P, o_block], f32, tag="ps", name="ps_0_0")
            for _ in range(30):
                nc.tensor.matmul(warm_ps[:, :P], warm_in[:], warm_in[:],
                                 start=True, stop=True)

            # Prologue: W block 0 and x interleaved per k-tile, then one
            # 8-bank MM group whose consumption rate matches DMA arrival.
            for k in range(KT):
                load_w(0, k)
                load_x(k)
            assert TT <= 8
            mm_group(0, 0, TT, first_ps=warm_ps)

            for ob in range(1, OB):
                for k in range(KT):
                    load_w(ob, k)
                for tg in range(TT // t_sub):
                    mm_group(ob, tg * t_sub, t_sub)

    nc.compile()
    return nc


def _get_nc(**kwargs):
    key = tuple(sorted(kwargs.items()))
    if key not in _NC_CACHE:
        _NC_CACHE[key] = _build_nc(**kwargs)
    return _NC_CACHE[key]


def _pack_w(W, o_block=512):
    """W [O, I] f32 -> o-block-major W^T [O//o_block, I, o_block], bf16.

    Only sign(W) is consumed on-device; the f32->bf16 cast preserves the
    sign of every value exactly (bf16 has f32's exponent range, so no
    nonzero f32 rounds to bf16 zero), making this a lossless wire encoding
    of the used information at half the DMA cost.
    """
    import ml_dtypes

    O, I = W.shape
    wt = W.T  # [I, O] view
    return np.ascontiguousarray(
        wt.reshape(I, O // o_block, o_block).transpose(1, 0, 2)
    ).astype(ml_dtypes.bfloat16)


def kernel(x, W):
    import os

    from concourse.bass_utils import run_bass_kernel_spmd

    global LAST_RESULTS

    # A stray BASS_TRACE in the environment would route run_bass_kernel_spmd
    # through the NTFF profiling hook, which needs antenv.axon_hooks; if
    # that module isn't importable here, neutralize tracing instead of
    # crashing.
    try:
        import antenv.axon_hooks  # noqa: F401
    except ImportError:
        os.environ.setdefault("BASS_NEVER_TRACE", "1")

    x = np.ascontiguousarray(np.asarray(x), dtype=np.float32)
    W = np.ascontiguousarray(np.asarray(W), dtype=np.float32)
    assert x.shape == (TOKENS, IN_F), x.shape
    assert W.shape == (OUT_F, IN_F), W.shape

    T = TOKENS // N_CORES
    nc = _get_nc()

    wtb = _pack_w(W)
    in_maps = [
        {
            "xt": np.ascontiguousarray(x[c * T:(c + 1) * T].T)
                    .astype(np.float16),
            "wtb": wtb,
        }
        for c in range(N_CORES)
    ]

    # Device executions can transiently fail (NRT_EXEC_UNIT_UNRECOVERABLE
    # observed once in ~10 runs); re-dispatching recovers, so retry.
    import time

    last_exc = None
    for attempt in range(3):
        try:
            res = run_bass_kernel_spmd(
                nc, in_maps, core_ids=list(range(N_CORES))
            )
            break
        except Exception as e:  # noqa: BLE001
            last_exc = e
            time.sleep(5 * (attempt + 1))
    else:
        raise last_exc

    LAST_RESULTS = res
    return np.concatenate([r["y"] for r in res.results], axis=0)


# revision 28
# speedup vs baseline: 1.0042x; 1.0042x over previous
"""Trainium2 Bass kernel for BinarizedLinear: y = x @ sign(W)^T.

Full-input contract: kernel(x, W) takes the unsharded inputs
(x: [8192, 4096] f32, W: [4096, 4096] f32) and returns y: [8192, 4096] f32.

Distribution: data-parallel over tokens. Each of the 8 NeuronCores gets a
[1024, 4096] token shard of x plus a full replica of W, computes
y_shard = x_shard @ sign(W)^T, and the shards are concatenated on the host.

Device kernel (per core):
  - sign(W) is computed on the Scalar (ACT) engine; the {-1, 0, +1} values
    are exact in fp16, so the matmul runs at the 16-bit TensorE rate
    (4x the fp32 rate). The only lossy step is x's f32->f16 rounding
    (~2^-11 relative per element), done while marshaling the shard.
  - Matmuls contract over in_features (on SBUF partitions), accumulating
    32 k-tiles into PSUM in fp32. The first out-feature block uses all 8
    PSUM banks; later blocks use 4+4 so one group's accumulation overlaps
    the other's drain. Junk matmuls during the data-less startup window
    warm the PE's HAM clock gate to 2.4GHz before real work arrives.
  - Host supplies transposed layouts (x^T per shard in fp16, W^T in
    o-block-major [OB, I, 512] bf16) so every DMA is a single linear
    transfer and the contraction dim lands on SBUF partitions with no
    on-device transposes. DMA engine choice (sync vs scalar HWDGE queues)
    plus pool-gated prefetch depth order HBM traffic by need.

Measured: 462.9us on hardware vs the 437.4us fp16 TensorE roofline for
this shape (94.5%); steady-state matmul cadence is the theoretical 216ns.
"""

import numpy as np

TOKENS, IN_F, OUT_F = 8192, 4096, 4096
N_CORES = 8

LAST_RESULTS = None  # BassKernelResults of the most recent run (for profiling)
_NC_CACHE = {}


def _build_nc(T=TOKENS // N_CORES, I=IN_F, O=OUT_F, o_block=512, t_sub=4):
    """Build + compile the per-core Bass module.

    DRAM tensors (per core):
      xt:  [I, T] f16            -- x_shard^T (compute precision)
      wtb: [OB, I, o_block] bf16 -- W^T, o-block-major (sign-exact wire)
      y:   [T, O] f32
    """
    import concourse.mybir as mybir
    import concourse.tile as tile
    from concourse import bacc

    f32, f16 = mybir.dt.float32, mybir.dt.float16
    bf16 = mybir.dt.bfloat16

    P = 128
    KT = I // P          # k-tiles (contraction)
    OB = O // o_block    # output-feature blocks
    TT = T // P          # token tiles
    assert I % P == 0 and O % o_block == 0 and T % P == 0 and TT % t_sub == 0

    nc = bacc.Bacc(
        "TRN2", target_bir_lowering=False, debug=False, enable_asserts=False
    )
    # x^T travels in fp16 — the kernel's compute precision. The host-side
    # f32->f16 rounding is identical to the f32->f16 cast the device would
    # otherwise perform on arrival (same RNE), so device math and output
    # are bit-identical; shipping the compute format halves x DMA.
    xt = nc.dram_tensor("xt", [I, T], f16, kind="ExternalInput")
    # W^T travels as bf16: bf16 keeps f32's exponent range, so the cast
    # preserves sign exactly (no nonzero value rounds to zero); only sign(W)
    # is consumed, so this is a lossless encoding of the used information.
    wtb = nc.dram_tensor("wtb", [OB, I, o_block], bf16, kind="ExternalInput")
    y = nc.dram_tensor("y", [T, O], f32, kind="ExternalOutput")

    xt3 = xt.ap().rearrange("(k p) t -> k p t", p=P)       # [KT, 128, T]
    wt4 = wtb.ap().rearrange("b (k p) o -> b k p o", p=P)  # [OB, KT, 128, o_block]
    y3 = y.ap().rearrange("(t p) o -> t p o", p=P)         # [TT, 128, O]

    with tile.TileContext(nc) as tc:
        with (
            tc.tile_pool(name="xres", bufs=KT) as xres_pool,
            tc.tile_pool(name="wstage", bufs=6) as wstage_pool,
            tc.tile_pool(name="wb", bufs=KT + 16) as wb_pool,
            tc.tile_pool(name="ystage", bufs=6) as ystage_pool,
            tc.tile_pool(name="psum", bufs=8, space="PSUM") as psum_pool,
        ):
            xf = [None] * KT
            wb = [None] * KT

            def load_x(k):
                xx = xres_pool.tile([P, T], f16, tag="xres", name=f"xf_{k}")
                nc.sync.dma_start(xx[:], xt3[k])
                xf[k] = xx

            def load_w(ob, k):
                st = wstage_pool.tile([P, o_block], bf16, tag="wstage",
                                      name=f"ws_{ob}_{k}")
                # DMA engine choice orders HBM traffic by need: blocks 0/1
                # ride sync (block 0 interleaved with x, block 1 queued
                # behind x — deprioritized for exactly block 0's
                # bandwidth-critical window); blocks 2+ ride the Activation
                # engine's independent HWDGE queue set, with their prefetch
                # depth gated by the wb pool (KT+16 slots) so they cannot
                # creep into block 0's window either.
                dma_eng = nc.sync if ob <= 1 else nc.scalar
                dma_eng.dma_start(st[:], wt4[ob, k])
                wbk = wb_pool.tile([P, o_block], f16, tag="wb", name=f"wb_{ob}_{k}")
                nc.scalar.sign(wbk[:], st[:])
                wb[k] = wbk

            def mm_group(ob, t0, nt, first_ps=None):
                """Accumulate + drain output tiles for t-tiles t0..t0+nt-1."""
                osl = slice(ob * o_block, (ob + 1) * o_block)
                psums = [
                    first_ps if (t == 0 and first_ps is not None) else
                    psum_pool.tile([P, o_block], f32, tag="ps",
                                   name=f"ps_{ob}_{t0 + t}")
                    for t in range(nt)
                ]
                for k in range(KT):
                    for t in range(nt):
                        ti = t0 + t
                        nc.tensor.matmul(
                            psums[t][:],
                            xf[k][:, ti * P:(ti + 1) * P],  # lhsT [K, M]
                            wb[k][:],                        # rhs  [K, N]
                            start=(k == 0),
                            stop=(k == KT - 1),
                        )
                last = (ob == OB - 1) and (t0 + nt == TT)
                for t in range(nt):
                    ti = t0 + t
                    yt = ystage_pool.tile([P, o_block], f32, tag="ystage",
                                          name=f"yt_{ob}_{ti}")
                    # Final group: split drains across DVE and ACT so the
                    # kernel tail isn't serialized on one engine.
                    if last and t % 2 == 1:
                        nc.scalar.copy(yt[:], psums[t][:])
                    else:
                        nc.vector.tensor_copy(yt[:], psums[t][:])
                    nc.sync.dma_start(y3[ti][:, osl], yt[:])

            # With fp16 x on the wire, block 0 is PE-bound, so matmuls that
            # run at the cold 1.2GHz HAM clock cost end-to-end time. Warm
            # the clock gate during the data-less startup window (~4-9us)
            # with junk matmuls on a zeroed tile; they land in the first
            # group's first PSUM bank, which the real k=0 matmul's
            # start=True resets.
            warm_in = wb_pool.tile([P, P], f16, tag="warm", bufs=1,
                                   name="warm_in")
            nc.gpsimd.memset(warm_in[:], 0.0)
            warm_ps = psum_pool.tile([P, o_block], f32, tag="ps", name="ps_0_0")
            for _ in range(64):
                nc.tensor.matmul(warm_ps[:, :P], warm_in[:], warm_in[:],
                                 start=True, stop=True)

            # Prologue: W block 0 and x interleaved per k-tile, then one
            # 8-bank MM group whose consumption rate matches DMA arrival.
            for k in range(KT):
                load_w(0, k)
                load_x(k)
            assert TT <= 8
            mm_group(0, 0, TT, first_ps=warm_ps)

            for ob in range(1, OB):
                for k in range(KT):
                    load_w(ob, k)
                for tg in range(TT // t_sub):
                    mm_group(ob, tg * t_sub, t_sub)

    nc.compile()
    return nc


def _get_nc(**kwargs):
    key = tuple(sorted(kwargs.items()))
    if key not in _NC_CACHE:
        _NC_CACHE[key] = _build_nc(**kwargs)
    return _NC_CACHE[key]


def _pack_w(W, o_block=512):
    """W [O, I] f32 -> o-block-major W^T [O//o_block, I, o_block], bf16.

    Only sign(W) is consumed on-device; the f32->bf16 cast preserves the
    sign of every value exactly (bf16 has f32's exponent range, so no
    nonzero f32 rounds to bf16 zero), making this a lossless wire encoding
    of the used information at half the DMA cost.
    """
    import ml_dtypes

    O, I = W.shape
    wt = W.T  # [I, O] view
    return np.ascontiguousarray(
        wt.reshape(I, O // o_block, o_block).transpose(1, 0, 2)
    ).astype(ml_dtypes.bfloat16)


def kernel(x, W):
    import os

    from concourse.bass_utils import run_bass_kernel_spmd

    global LAST_RESULTS

    # A stray BASS_TRACE in the environment would route run_bass_kernel_spmd
    # through the NTFF profiling hook, which needs antenv.axon_hooks; if
    # that module isn't importable here, neutralize tracing instead of
    # crashing.
    try:
        import antenv.axon_hooks  # noqa: F401
    except ImportError:
        os.environ.setdefault("BASS_NEVER_TRACE", "1")

    x = np.ascontiguousarray(np.asarray(x), dtype=np.float32)
    W = np.ascontiguousarray(np.asarray(W), dtype=np.float32)
    assert x.shape == (TOKENS, IN_F), x.shape
    assert W.shape == (OUT_F, IN_F), W.shape

    T = TOKENS // N_CORES
    nc = _get_nc()

    wtb = _pack_w(W)
    in_maps = [
        {
            "xt": np.ascontiguousarray(x[c * T:(c + 1) * T].T)
                    .astype(np.float16),
            "wtb": wtb,
        }
        for c in range(N_CORES)
    ]

    # Device executions can transiently fail (NRT_EXEC_UNIT_UNRECOVERABLE
    # observed once in ~10 runs); re-dispatching recovers, so retry.
    import time

    last_exc = None
    for attempt in range(3):
        try:
            res = run_bass_kernel_spmd(
                nc, in_maps, core_ids=list(range(N_CORES))
            )
            break
        except Exception as e:  # noqa: BLE001
            last_exc = e
            time.sleep(5 * (attempt + 1))
    else:
        raise last_exc

    LAST_RESULTS = res
    return np.concatenate([r["y"] for r in res.results], axis=0)
